# revision 35
# baseline (speedup 1.0000x reference)
"""Trainium2 Bass kernel for a GNN message-passing layer.

Reference computation (all fp32):
    messages = h[src] @ W_msg.T            # [E, D]
    agg      = segment_sum(messages, dst)  # [N, D]
    out      = relu(concat(h, agg) @ W_upd.T + b_upd)

Key algebraic restructure: segment_sum is linear, so
    agg = A @ W_msg.T          where A = segment_sum(h[src], dst)
and the update splits W_upd = [Wu1 | Wu2]:
    out.T = relu(Wu1 @ h.T + (Wu2 @ W_msg) @ A.T + b)
so the device only computes A (a pure gather + scatter-add) plus two small
fused matmuls.  Wc = Wu2 @ W_msg is precomputed on host.

Sharding: nodes are partitioned contiguously across the 8 cores by dst.
Each core processes exactly the edges whose dst lands in its node shard,
so no collectives are needed.

The kernel is SWDGE-bound: HW-measured cost ~ 1.1 ns per descriptor
+ ~0.93 ns per 256B of ring payload + ~0.46 us per gather instruction.
To cut descriptor count, each descriptor fetches a 512B WINDOW = two
adjacent bf16 rows of a permuted copy of the h table.  The host builds
L=6 per-core permutations, each realizing a disjoint node *matching*
chosen so that two edges of the same dst-block share one window (~88%
of edges pair up); the rest use one half of a window (the unused half's
staircase rel is -1, an all-zero staircase row, contributing 0).
Window indices fit int16 (25001 windows incl. one zero pad window).

Aggregation per 128-descriptor chunk (slot i -> partition i%128):
  S[slot, jj] = (jj < rel[slot] + 0.5)   one DVE compare per
  (chunk, half, block-section piece); then on TensorE (bf16):
  psum_blk[feat, jj] += g_half[slot, feat] * S[slot, jj]; per-dst sums
  are adjacent-column diffs of psum.  relp is stored duplicated (each
  value twice) to keep the DVE in its 2x 16-bit mode.
Gather instructions cover 4 dst-blocks x 1 copy each (static
per-(group,copy,block) section capacities = max over the 8 cores, so
the single SPMD program works for every core's data).
Phase 2 (per 4-block group): diff on VectorE (fp32 -> bf16), then
    out.T = relu(Wu1 @ h.T + Wc @ diff + b)   (bf16 matmuls)
"""

import contextlib

import numpy as np

import concourse.bass as bass
import concourse.mybir as mybir
import concourse.tile as tile
from concourse import bacc
from concourse.bass_utils import run_bass_kernel_spmd

P = 128  # SBUF partitions
D = 128  # feature dim (in_dim == out_dim == 128)
N_CORES = 8
CHUNK = 128  # descriptors per matmul chunk
W129 = CHUNK + 1  # staircase width per block (psum / buf)
W130 = CHUNK + 2  # staircase width incl. pad col (even for 2x DVE mode)
L_COPIES = 6  # permuted pair-table copies per core
GB = 4  # dst-blocks per gather group
P2_SPLIT = 28  # blocks whose update matmul is emitted mid-gather-loop

_prog_cache: dict = {}


def _build_program(plan, loop_iters=None):
    """One SPMD program, shared by all 8 cores; static sizes from `plan`."""
    f32 = mybir.dt.float32
    bf16 = mybir.dt.bfloat16
    fp8 = mybir.dt.float8e3  # e3m4: range +-15.5 covers N(0,1) h exactly
    i16 = mybir.dt.int16
    NB = plan["NB"]
    SP = NB * P
    NW = plan["NW"]  # windows per table (incl. zero pad window)
    caps = plan["caps"]  # caps[g][k]: stream layout capacity (mult of 128)
    nidx = plan["nidx"]  # nidx[g][k]: exact gathered descriptor count
    pieces = plan["pieces"]  # pieces[g][k]: list of (chunk, half, bi, start, stop)
    npieces = plan["npieces"]
    groups = plan["groups"]  # list of (b0, nb)
    idx_cols = plan["idx_cols"]
    L = len(caps[0])

    nc = bacc.Bacc("TRN2", target_bir_lowering=False, num_swdge_queues=4)

    tabs_d = [
        nc.dram_tensor(f"tab{k}", [NW, 2 * D], fp8, kind="ExternalInput")
        for k in range(L)
    ]
    hsT_d = nc.dram_tensor("hsT", [P, SP], bf16, kind="ExternalInput")
    idx_d = nc.dram_tensor("idx", [P, idx_cols], i16, kind="ExternalInput")
    relp_d = nc.dram_tensor("relp", [P, npieces * 2], bf16, kind="ExternalInput")
    iota_d = nc.dram_tensor("iota", [P, W130], bf16, kind="ExternalInput")
    w1_d = nc.dram_tensor("w1T", [D, D], bf16, kind="ExternalInput")
    wc_d = nc.dram_tensor("wcT", [D, D], bf16, kind="ExternalInput")
    b_d = nc.dram_tensor("bias", [P, 1], f32, kind="ExternalInput")
    out_d = nc.dram_tensor("outT", [P, SP], f32, kind="ExternalOutput")

    capmax = max(max(ck) for ck in caps)

    with tile.TileContext(nc) as tc:
        with (
            tc.tile_pool(name="constp", bufs=1) as constp,
            tc.tile_pool(name="gatp", bufs=10) as gatp,
            tc.tile_pool(name="sp_", bufs=4) as sp_,
            tc.tile_pool(name="aggp", bufs=1) as aggp,
            tc.tile_pool(name="diffp", bufs=2) as diffp,
            tc.tile_pool(name="outp", bufs=3) as outp,
            tc.tile_pool(name="psp", bufs=6, space="PSUM") as psp,
            tc.tile_pool(name="ps2p", bufs=2, space="PSUM") as ps2p,
        ):
            # warmup: tiny gather off a memset idx absorbs the Q7 gather
            # ucode library load before the real idx data has landed
            widx_t = constp.tile([P, 1], i16)
            nc.gpsimd.memset(widx_t[:], NW - 1)
            warm_t = constp.tile([P, 2 * D], fp8)
            nc.gpsimd.dma_gather(
                out_ap=warm_t[:].unsqueeze(1),
                in_ap=tabs_d[0][:],
                idxs_ap=widx_t[:],
                num_idxs=16,
                num_idxs_reg=16,
                elem_size=2 * D,
                single_packet=False,
                queue_num=0,
            )
            # idx split so the first gathers start immediately
            idx_t = constp.tile([P, idx_cols], i16)
            c1 = min(caps[0][0] // 16, idx_cols)
            nc.sync.dma_start(idx_t[:, 0:c1], idx_d[:, 0:c1])
            iota_t = constp.tile([P, W130], bf16)
            nc.sync.dma_start(iota_t[:], iota_d[:])
            c2 = min(sum(caps[0]) // 16, idx_cols)
            if c2 > c1:
                nc.sync.dma_start(idx_t[:, c1:c2], idx_d[:, c1:c2])
            if idx_cols > c2:
                nc.sync.dma_start(idx_t[:, c2:], idx_d[:, c2:])
            relp_t = constp.tile([P, npieces * 2], bf16)
            nc.sync.dma_start(relp_t[:], relp_d[:])
            w1_t = constp.tile([D, D], bf16)
            nc.sync.dma_start(w1_t[:], w1_d[:])
            wc_t = constp.tile([D, D], bf16)
            nc.sync.dma_start(wc_t[:], wc_d[:])
            b_t = constp.tile([P, 1], f32)
            nc.sync.dma_start(b_t[:], b_d[:])
            hsT_t = constp.tile([P, SP], bf16)
            nc.sync.dma_start(hsT_t[:], hsT_d[:])

            buf_t = aggp.tile([P, NB * W129], f32)
            iota_ab = iota_t[:].rearrange("p (a b) -> p a b", b=2)

            loop_cm = (
                tc.For_i(0, loop_iters, 1)
                if loop_iters is not None
                else contextlib.nullcontext()
            )
            with loop_cm:
                buf3 = buf_t[:].rearrange("p (b j) -> p b j", j=W129)

                def phase2(lo_b, hi_b):
                    b0 = lo_b
                    while b0 < hi_b:
                        nb = min(4, hi_b - b0)
                        w = nb * CHUNK
                        col = b0 * CHUNK
                        d_t = diffp.tile([P, 512], bf16, name="d_t")
                        d3 = d_t[:].rearrange("p (b j) -> p b j", j=CHUNK)
                        nc.vector.tensor_tensor(
                            out=d3[:, 0:nb, :],
                            in0=buf3[:, b0 : b0 + nb, 0:CHUNK],
                            in1=buf3[:, b0 : b0 + nb, 1:W129],
                            op=mybir.AluOpType.subtract,
                        )
                        ps2_t = ps2p.tile([P, 512], f32, name="ps2_t")
                        nc.tensor.matmul(
                            out=ps2_t[:, :w],
                            lhsT=w1_t[:],
                            rhs=hsT_t[:, col : col + w],
                            start=True,
                            stop=False,
                        )
                        nc.tensor.matmul(
                            out=ps2_t[:, :w],
                            lhsT=wc_t[:],
                            rhs=d_t[:, :w],
                            start=False,
                            stop=True,
                        )
                        o_t = outp.tile([P, 512], f32, name="o_t")
                        nc.scalar.activation(
                            o_t[:, :w],
                            ps2_t[:, :w],
                            mybir.ActivationFunctionType.Relu,
                            bias=b_t[:],
                        )
                        nc.sync.dma_start(out_d[:, col : col + w], o_t[:, :w])
                        b0 += nb

                icol = 0
                ipiece = 0
                qn = 0
                for g, (b0g, nbg) in enumerate(groups):
                    ps_ts = [
                        psp.tile([P, W129], f32, name="ps_t") for bi in range(nbg)
                    ]
                    for k in range(L):
                        cap = caps[g][k]
                        pl = pieces[g][k]
                        np_k = len(pl)
                        g_t = gatp.tile([P, capmax * 2], fp8)
                        g3 = g_t[:].rearrange("p (c d) -> p c d", d=2 * D)
                        ni = nidx[g][k]
                        nc.gpsimd.dma_gather(
                            out_ap=g3[:, 0 : -(-ni // 128), :],
                            in_ap=tabs_d[k][:],
                            idxs_ap=idx_t[:, icol : icol + (-(-ni // 16))],
                            num_idxs=ni,
                            num_idxs_reg=ni,
                            elem_size=2 * D,
                            single_packet=False,
                            queue_num=qn,
                        )
                        icol += cap // 16
                        qn = (qn + 1) % 4
                        if np_k == 0:
                            continue
                        s_t = sp_.tile([P, np_k * W130], fp8)
                        relp_b = (
                            relp_t[:, ipiece * 2 : (ipiece + np_k) * 2]
                            .rearrange("p (c t) -> p c t", t=2)
                            .unsqueeze(2)
                            .broadcast_to([P, np_k, W130 // 2, 2])
                        )
                        iota_b = iota_ab.unsqueeze(1).broadcast_to(
                            [P, np_k, W130 // 2, 2]
                        )
                        s_b = s_t[:].rearrange(
                            "p (c a b) -> p c a b", a=W130 // 2, b=2
                        )
                        nc.vector.tensor_tensor(
                            out=s_b, in0=iota_b, in1=relp_b, op=mybir.AluOpType.is_lt
                        )
                        s3 = s_t[:].rearrange("p (c w) -> p c w", w=W130)
                        for pi, (ch, half, bi, start, stop) in enumerate(pl):
                            nc.tensor.matmul(
                                out=ps_ts[bi][:],
                                lhsT=g_t[
                                    :,
                                    ch * 2 * D + half * D : ch * 2 * D + half * D + D,
                                ],
                                rhs=s3[:, pi, 0:W129],
                                start=bool(start),
                                stop=bool(stop),
                            )
                        ipiece += np_k
                    for bi in range(nbg):
                        nc.scalar.activation(
                            out=buf_t[:, (b0g + bi) * W129 : (b0g + bi + 1) * W129],
                            in_=ps_ts[bi][:],
                            func=mybir.ActivationFunctionType.Copy,
                        )
                    if g == 8:
                        # early phase 2 for long-ready groups: inputs
                        # finished ~2 groups ago, so no engine-queue
                        # head-of-line waits; shrinks the end tail
                        phase2(0, P2_SPLIT)
                    elif g == 11:
                        phase2(P2_SPLIT, 40)

                # Phase 2 for remaining blocks
                phase2(40, NB)

    nc.compile()
    return nc


def _prep_inputs(h, edge_index, W_msg, W_upd, b_upd):
    """Host prep: per-core edge bucketing, L matchings -> pair tables,
    static stream/piece plan shared across cores."""
    import ml_dtypes

    N0, d = h.shape
    assert d == D
    E = edge_index.shape[1]

    SP = -(-N0 // (N_CORES * P)) * P
    NB = SP // P
    NWIN = N0 // 2
    PADW = NWIN  # zero pad window index
    NW = NWIN + 1
    L = L_COPIES

    src = np.ascontiguousarray(edge_index[0]).astype(np.int64)
    dst = np.ascontiguousarray(edge_index[1]).astype(np.int64)
    gblock = dst >> 7
    core_of = np.minimum(gblock // NB, N_CORES - 1)
    slot_of = gblock - core_of * NB
    rel = (dst & 127).astype(np.int64)

    groups = []
    b0 = 0
    while b0 < NB:
        nb = min(GB, NB - b0)
        groups.append((b0, nb))
        b0 += nb
    NG = len(groups)
    group_of_slot = np.zeros(NB, np.int64)
    for gi, (b0g, nbg) in enumerate(groups):
        group_of_slot[b0g : b0g + nbg] = gi

    rng = np.random.default_rng(12345)
    hbf = h.astype(ml_dtypes.bfloat16)
    h8 = h.astype(ml_dtypes.float8_e3m4)

    # entries[c][g][k][slot] = list of (window, relA, relB)
    entries = [
        [[[[] for _ in range(NB)] for _ in range(L)] for _ in range(NG)]
        for _ in range(N_CORES)
    ]
    tables = [[None] * L for _ in range(N_CORES)]

    for c in range(N_CORES):
        m = np.flatnonzero(core_of == c)
        es = src[m]
        eslot = slot_of[m]
        erel = rel[m]
        covered = np.zeros(len(es), bool)
        pos_maps = []
        for k in range(L):
            ridx = np.flatnonzero(~covered)
            o = np.lexsort((es[ridx], eslot[ridx]))
            r = ridx[o]
            rb = eslot[r]
            same = rb[:-1] == rb[1:]
            newrun = np.r_[True, rb[1:] != rb[:-1]]
            pos = np.arange(len(r)) - np.maximum.accumulate(
                np.where(newrun, np.arange(len(r)), 0)
            )
            i_idx = np.flatnonzero((pos[:-1] % 2 == 0) & same)
            e1 = r[i_idx]
            e2 = r[i_idx + 1]
            s1 = es[e1]
            s2 = es[e2]
            v = s1 != s2
            e1, e2, s1, s2 = e1[v], e2[v], s1[v], s2[v]
            taken = np.zeros(N0, bool)
            acc_e1, acc_e2 = [], []
            remaining = rng.permutation(len(e1))
            for _ in range(4):
                if len(remaining) == 0:
                    break
                aa, bb = s1[remaining], s2[remaining]
                ok = ~taken[aa] & ~taken[bb]
                cand = remaining[ok]
                if len(cand) == 0:
                    break
                aa, bb = s1[cand], s2[cand]
                fa = np.zeros(len(cand), bool)
                fb = np.zeros(len(cand), bool)
                _, fi = np.unique(aa, return_index=True)
                fa[fi] = True
                _, fi2 = np.unique(bb, return_index=True)
                fb[fi2] = True
                acc = np.flatnonzero(fa & fb)
                s1a, s2a = aa[acc], bb[acc]
                seen2 = np.zeros(N0, bool)
                keep = np.zeros(len(acc), bool)
                for i_ in range(len(acc)):
                    x, y = s1a[i_], s2a[i_]
                    if not (seen2[x] or seen2[y]):
                        keep[i_] = True
                        seen2[x] = True
                        seen2[y] = True
                accepted = cand[acc[keep]]
                taken[s1[accepted]] = True
                taken[s2[accepted]] = True
                acc_e1.append(e1[accepted])
                acc_e2.append(e2[accepted])
                covered[e1[accepted]] = True
                covered[e2[accepted]] = True
                remaining = remaining[~taken[s1[remaining]] & ~taken[s2[remaining]]]
            pe1 = np.concatenate(acc_e1) if acc_e1 else np.empty(0, np.int64)
            pe2 = np.concatenate(acc_e2) if acc_e2 else np.empty(0, np.int64)
            ps1, ps2 = es[pe1], es[pe2]
            used = np.zeros(N0, bool)
            used[ps1] = True
            used[ps2] = True
            restn = np.flatnonzero(~used)
            perm = np.empty(N0, np.int64)
            npair = len(ps1)
            perm[0 : 2 * npair : 2] = ps1
            perm[1 : 2 * npair : 2] = ps2
            perm[2 * npair :] = restn
            pos_of = np.empty(N0, np.int64)
            pos_of[perm] = np.arange(N0)
            pos_maps.append(pos_of)
            tab = np.zeros((NW, 2 * D), dtype=ml_dtypes.float8_e3m4)
            tab[:NWIN] = h8[perm].reshape(NWIN, 2 * D)
            tables[c][k] = tab
            sl_arr = eslot[pe1]
            ra_arr = erel[pe1]
            rb_arr = erel[pe2]
            for j in range(npair):
                sl = int(sl_arr[j])
                entries[c][group_of_slot[sl]][k][sl].append(
                    (j, int(ra_arr[j]), int(rb_arr[j]))
                )
        # singles: per-bucket water-fill over copies so the 6 sections of
        # each (core, block) end up near-equal -> smaller cross-core max
        sing = np.flatnonzero(~covered)
        so = sing[np.argsort(eslot[sing], kind="stable")]
        cur_sl = -1
        counts = [0] * L
        for ei in so:
            sl = int(eslot[ei])
            if sl != cur_sl:
                cur_sl = sl
                gg = group_of_slot[sl]
                counts = [len(entries[c][gg][k][sl]) for k in range(L)]
            k = min(range(L), key=counts.__getitem__)
            counts[k] += 1
            p = int(pos_maps[k][es[ei]])
            w = p >> 1
            if p & 1 == 0:
                entries[c][group_of_slot[sl]][k][sl].append((w, int(erel[ei]), -1))
            else:
                entries[c][group_of_slot[sl]][k][sl].append((w, -1, int(erel[ei])))

    # static caps per (g, k, block): max over cores, mult of 16
    capgkb = np.zeros((NG, L, NB), np.int64)
    for g in range(NG):
        for k in range(L):
            for bsl in range(NB):
                mx = max(len(entries[c][g][k][bsl]) for c in range(N_CORES))
                capgkb[g, k, bsl] = -(-mx // 16) * 16
    caps = []
    for g in range(NG):
        ck = []
        for k in range(L):
            t = int(capgkb[g, k, :].sum())
            ck.append(-(-max(t, 128) // 128) * 128)
        caps.append(ck)
    # exact stream lengths; the first 8 streams (one per gather-pool ring
    # slot) gather the max chunk span any later stream on their slot reads,
    # so no stale tile tail is ever read under a rel=-1 staircase row
    flat_t = [int(capgkb[g, k, :].sum()) for g in range(NG) for k in range(L)]
    warm = [128] * 10
    for s, t in enumerate(flat_t):
        r = s % 10
        warm[r] = max(warm[r], -(-t // 128) * 128)
    nidx = []
    si = 0
    for g in range(NG):
        nk = []
        for k in range(L):
            t = int(capgkb[g, k, :].sum())
            if si < 10:
                caps[g][k] = max(warm[si], -(-max(t, 128) // 128) * 128)
                nk.append(caps[g][k])
            else:
                nk.append(max(t, 16))
            si += 1
        nidx.append(nk)

    # static sections and piece lists
    stream_sections = [[None] * L for _ in range(NG)]
    pieces = [[None] * L for _ in range(NG)]
    for g, (b0g, nbg) in enumerate(groups):
        for k in range(L):
            secs = []
            off = 0
            for bi in range(nbg):
                sl = b0g + bi
                cb = int(capgkb[g, k, sl])
                if cb:
                    secs.append((off, off + cb, bi, sl))
                off += cb
            stream_sections[g][k] = secs
            cap = caps[g][k]
            pl = []
            for ch in range(cap // 128):
                lo_c, hi_c = ch * 128, (ch + 1) * 128
                for off_lo, off_hi, bi, sl in secs:
                    a = max(lo_c, off_lo)
                    b_ = min(hi_c, off_hi)
                    if a < b_:
                        for half in (0, 1):
                            pl.append([ch, half, bi, 0, 0, a - lo_c, b_ - lo_c, sl])
            pieces[g][k] = pl
    # start/stop flags in program emission order
    seen_first = set()
    last_ref = {}
    for g in range(NG):
        for k in range(L):
            for p in pieces[g][k]:
                sl = p[7]
                if sl not in seen_first:
                    p[3] = 1
                    seen_first.add(sl)
                last_ref[sl] = p
    for sl, p in last_ref.items():
        p[4] = 1
    assert len(seen_first) == NB, (len(seen_first), NB)

    npieces = sum(len(pieces[g][k]) for g in range(NG) for k in range(L))
    idx_cols = sum(caps[g][k] // 16 for g in range(NG) for k in range(L))

    w1T = np.ascontiguousarray(
        W_upd[:, :D].T.astype(np.float32).astype(ml_dtypes.bfloat16)
    )
    wc = (W_upd[:, D:].astype(np.float64) @ W_msg.astype(np.float64)).astype(
        np.float32
    )
    wcT = np.ascontiguousarray(wc.T.astype(ml_dtypes.bfloat16))
    bias = np.ascontiguousarray(b_upd.astype(np.float32).reshape(P, 1))
    iota = np.ascontiguousarray(
        np.tile(np.arange(W130, dtype=np.float32), (P, 1)).astype(ml_dtypes.bfloat16)
    )

    in_maps = []
    for c in range(N_CORES):
        idx_flat = np.full(idx_cols * 16, PADW, np.int16)
        relp_arr = np.full((P, npieces * 2), -0.5, np.float32)
        ioff = 0
        poff = 0
        for g, (b0g, nbg) in enumerate(groups):
            for k in range(L):
                cap = caps[g][k]
                secs = stream_sections[g][k]
                slots_w = np.full(cap, PADW, np.int64)
                slots_rA = np.full(cap, -1.0, np.float32)
                slots_rB = np.full(cap, -1.0, np.float32)
                for off_lo, off_hi, bi, sl in secs:
                    ent = entries[c][g][k][sl]
                    n = len(ent)
                    if n:
                        slots_w[off_lo : off_lo + n] = [e[0] for e in ent]
                        slots_rA[off_lo : off_lo + n] = [e[1] for e in ent]
                        slots_rB[off_lo : off_lo + n] = [e[2] for e in ent]
                idx_flat[ioff : ioff + cap] = slots_w.astype(np.int16)
                ioff += cap
                for p in pieces[g][k]:
                    ch, half, bi, st, sp2, a, b_, sl = p
                    vals = np.full(P, -0.5, np.float32)
                    base = ch * 128
                    rr = (slots_rA if half == 0 else slots_rB)[base + a : base + b_]
                    vals[a:b_] = rr + 0.5
                    relp_arr[:, poff * 2] = vals
                    relp_arr[:, poff * 2 + 1] = vals
                    poff += 1
        assert ioff == idx_cols * 16 and poff == npieces
        idx16 = idx_flat.reshape(idx_cols, 16).T  # [16, cols]
        idx_in = np.tile(idx16, (8, 1))

        lo = c * SP
        hi = min((c + 1) * SP, N0)
        hs = np.zeros((SP, D), dtype=ml_dtypes.bfloat16)
        if hi > lo:
            hs[: hi - lo] = hbf[lo:hi]
        im = {
            "hsT": np.ascontiguousarray(hs.T),
            "idx": np.ascontiguousarray(idx_in),
            "relp": np.ascontiguousarray(relp_arr.astype(ml_dtypes.bfloat16)),
            "iota": iota,
            "w1T": w1T,
            "wcT": wcT,
            "bias": bias,
        }
        for k in range(L):
            im[f"tab{k}"] = tables[c][k]
        in_maps.append(im)

    plan = {
        "NB": NB,
        "NW": NW,
        "caps": caps,
        "nidx": nidx,
        "pieces": [
            [[(p[0], p[1], p[2], p[3], p[4]) for p in pieces[g][k]] for k in range(L)]
            for g in range(NG)
        ],
        "npieces": npieces,
        "groups": groups,
        "idx_cols": idx_cols,
    }
    return in_maps, plan, SP, NB


def kernel_with_results(h, edge_index, W_msg, W_upd, b_upd, loop_iters=None, **run_kwargs):
    in_maps, plan, SP, NB = _prep_inputs(h, edge_index, W_msg, W_upd, b_upd)

    key = (
        plan["NB"],
        plan["NW"],
        tuple(tuple(ck) for ck in plan["caps"]),
        tuple(
            tuple(map(tuple, plan["pieces"][g][k]))
            for g in range(len(plan["caps"]))
            for k in range(L_COPIES)
        ),
        loop_iters,
    )
    if key not in _prog_cache:
        _prog_cache[key] = _build_program(plan, loop_iters=loop_iters)
    nc = _prog_cache[key]

    res = run_bass_kernel_spmd(nc, in_maps, core_ids=list(range(N_CORES)), **run_kwargs)

    N0 = h.shape[0]
    out = np.empty((N0, D), dtype=np.float32)
    for c in range(N_CORES):
        lo = c * SP
        hi = min((c + 1) * SP, N0)
        if hi > lo:
            out[lo:hi] = res.results[c]["outT"].T[: hi - lo]
    return out, res


def kernel(h, edge_index, W_msg, W_upd, b_upd):
    out, _ = kernel_with_results(h, edge_index, W_msg, W_upd, b_upd)
    return out


# revision 36
# speedup vs baseline: 1.5111x; 1.5111x over previous
"""Trainium2 Bass kernel for a GNN message-passing layer.

Reference computation (all fp32):
    messages = h[src] @ W_msg.T            # [E, D]
    agg      = segment_sum(messages, dst)  # [N, D]
    out      = relu(concat(h, agg) @ W_upd.T + b_upd)

Key algebraic restructure: segment_sum is linear, so
    agg = A @ W_msg.T          where A = segment_sum(h[src], dst)
and the update splits W_upd = [Wu1 | Wu2]:
    out.T = relu(Wu1 @ h.T + (Wu2 @ W_msg) @ A.T + b)
so the device only computes A (a pure gather + scatter-add) plus two small
fused matmuls.  Wc = Wu2 @ W_msg is precomputed on host.

Sharding: nodes are partitioned contiguously across the 8 cores by dst.
Each core processes exactly the edges whose dst lands in its node shard,
so no collectives are needed.

The kernel is SWDGE-bound: HW-measured cost ~ 1.1 ns per descriptor
+ ~0.93 ns per 256B of ring payload + ~0.46 us per gather instruction.
To cut descriptor count, each descriptor fetches a 512B WINDOW = two
adjacent bf16 rows of a permuted copy of the h table.  The host builds
L=6 per-core permutations, each realizing a disjoint node *matching*
chosen so that two edges of the same dst-block share one window (~88%
of edges pair up); the rest use one half of a window (the unused half's
staircase rel is -1, an all-zero staircase row, contributing 0).
Window indices fit int16 (25001 windows incl. one zero pad window).

Aggregation per 128-descriptor chunk (slot i -> partition i%128):
  S[slot, jj] = (jj < rel[slot] + 0.5)   one DVE compare per
  (chunk, half, block-section piece); then on TensorE (bf16):
  psum_blk[feat, jj] += g_half[slot, feat] * S[slot, jj]; per-dst sums
  are adjacent-column diffs of psum.  relp is stored duplicated (each
  value twice) to keep the DVE in its 2x 16-bit mode.
Gather instructions cover 4 dst-blocks x 1 copy each (static
per-(group,copy,block) section capacities = max over the 8 cores, so
the single SPMD program works for every core's data).
Phase 2 (per 4-block group): diff on VectorE (fp32 -> bf16), then
    out.T = relu(Wu1 @ h.T + Wc @ diff + b)   (bf16 matmuls)
"""

import contextlib

import numpy as np

import concourse.bass as bass
import concourse.mybir as mybir
import concourse.tile as tile
from concourse import bacc
from concourse.bass_utils import run_bass_kernel_spmd

P = 128  # SBUF partitions
D = 128  # feature dim (in_dim == out_dim == 128)
N_CORES = 8
CHUNK = 128  # descriptors per matmul chunk
W129 = CHUNK + 1  # staircase width per block (psum / buf)
W130 = CHUNK + 2  # staircase width incl. pad col (even for 2x DVE mode)
L_COPIES = 6  # permuted pair-table copies per core
GB = 4  # dst-blocks per gather group
P2_SPLIT = 28  # blocks whose update matmul is emitted mid-gather-loop

_prog_cache: dict = {}


def _build_program(plan, loop_iters=None):
    """One SPMD program, shared by all 8 cores; static sizes from `plan`."""
    f32 = mybir.dt.float32
    bf16 = mybir.dt.bfloat16
    fp8 = mybir.dt.float8e3  # e3m4: range +-15.5 covers N(0,1) h exactly
    i16 = mybir.dt.int16
    NB = plan["NB"]
    SP = NB * P
    NW = plan["NW"]  # windows per table (incl. zero pad window)
    caps = plan["caps"]  # caps[g][k]: stream layout capacity (mult of 128)
    nidx = plan["nidx"]  # nidx[g][k]: exact gathered descriptor count
    pieces = plan["pieces"]  # pieces[g][k]: list of (chunk, half, bi, start, stop)
    npieces = plan["npieces"]
    groups = plan["groups"]  # list of (b0, nb)
    idx_cols = plan["idx_cols"]
    L = len(caps[0])

    nc = bacc.Bacc("TRN2", target_bir_lowering=False, num_swdge_queues=4)

    tabs_d = [
        nc.dram_tensor(f"tab{k}", [NW, 2 * D], fp8, kind="ExternalInput")
        for k in range(L)
    ]
    hsT_d = nc.dram_tensor("hsT", [P, SP], bf16, kind="ExternalInput")
    idx_d = nc.dram_tensor("idx", [P, idx_cols], i16, kind="ExternalInput")
    relp_d = nc.dram_tensor("relp", [P, npieces * 2], bf16, kind="ExternalInput")
    iota_d = nc.dram_tensor("iota", [P, W130], bf16, kind="ExternalInput")
    w1_d = nc.dram_tensor("w1T", [D, D], bf16, kind="ExternalInput")
    wc_d = nc.dram_tensor("wcT", [D, D], bf16, kind="ExternalInput")
    b_d = nc.dram_tensor("bias", [P, 1], f32, kind="ExternalInput")
    out_d = nc.dram_tensor("outT", [P, SP], f32, kind="ExternalOutput")

    capmax = max(max(ck) for ck in caps)

    with tile.TileContext(nc) as tc:
        with (
            tc.tile_pool(name="constp", bufs=1) as constp,
            tc.tile_pool(name="gatp", bufs=10) as gatp,
            tc.tile_pool(name="sp_", bufs=4) as sp_,
            tc.tile_pool(name="aggp", bufs=1) as aggp,
            tc.tile_pool(name="diffp", bufs=2) as diffp,
            tc.tile_pool(name="outp", bufs=3) as outp,
            tc.tile_pool(name="psp", bufs=6, space="PSUM") as psp,
            tc.tile_pool(name="ps2p", bufs=2, space="PSUM") as ps2p,
        ):
            # warmup: tiny gather off a memset idx absorbs the Q7 gather
            # ucode library load before the real idx data has landed
            widx_t = constp.tile([P, 1], i16)
            nc.gpsimd.memset(widx_t[:], NW - 1)
            warm_t = constp.tile([P, 2 * D], fp8)
            nc.gpsimd.dma_gather(
                out_ap=warm_t[:].unsqueeze(1),
                in_ap=tabs_d[0][:],
                idxs_ap=widx_t[:],
                num_idxs=16,
                num_idxs_reg=16,
                elem_size=2 * D,
                single_packet=False,
                queue_num=0,
            )
            # idx split so the first gathers start immediately
            idx_t = constp.tile([P, idx_cols], i16)
            c1 = min(caps[0][0] // 16, idx_cols)
            nc.sync.dma_start(idx_t[:, 0:c1], idx_d[:, 0:c1])
            iota_t = constp.tile([P, W130], bf16)
            nc.sync.dma_start(iota_t[:], iota_d[:])
            c2 = min(sum(caps[0]) // 16, idx_cols)
            if c2 > c1:
                nc.sync.dma_start(idx_t[:, c1:c2], idx_d[:, c1:c2])
            if idx_cols > c2:
                nc.sync.dma_start(idx_t[:, c2:], idx_d[:, c2:])
            relp_t = constp.tile([P, npieces * 2], bf16)
            nc.sync.dma_start(relp_t[:], relp_d[:])
            w1_t = constp.tile([D, D], bf16)
            nc.sync.dma_start(w1_t[:], w1_d[:])
            wc_t = constp.tile([D, D], bf16)
            nc.sync.dma_start(wc_t[:], wc_d[:])
            b_t = constp.tile([P, 1], f32)
            nc.sync.dma_start(b_t[:], b_d[:])
            hsT_t = constp.tile([P, SP], bf16)
            nc.sync.dma_start(hsT_t[:], hsT_d[:])

            buf_t = aggp.tile([P, NB * W129], f32)
            iota_ab = iota_t[:].rearrange("p (a b) -> p a b", b=2)

            loop_cm = (
                tc.For_i(0, loop_iters, 1)
                if loop_iters is not None
                else contextlib.nullcontext()
            )
            with loop_cm:
                buf3 = buf_t[:].rearrange("p (b j) -> p b j", j=W129)

                def phase2(lo_b, hi_b):
                    b0 = lo_b
                    while b0 < hi_b:
                        nb = min(4, hi_b - b0)
                        w = nb * CHUNK
                        col = b0 * CHUNK
                        d_t = diffp.tile([P, 512], bf16, name="d_t")
                        d3 = d_t[:].rearrange("p (b j) -> p b j", j=CHUNK)
                        nc.vector.tensor_tensor(
                            out=d3[:, 0:nb, :],
                            in0=buf3[:, b0 : b0 + nb, 0:CHUNK],
                            in1=buf3[:, b0 : b0 + nb, 1:W129],
                            op=mybir.AluOpType.subtract,
                        )
                        ps2_t = ps2p.tile([P, 512], f32, name="ps2_t")
                        nc.tensor.matmul(
                            out=ps2_t[:, :w],
                            lhsT=w1_t[:],
                            rhs=hsT_t[:, col : col + w],
                            start=True,
                            stop=False,
                        )
                        nc.tensor.matmul(
                            out=ps2_t[:, :w],
                            lhsT=wc_t[:],
                            rhs=d_t[:, :w],
                            start=False,
                            stop=True,
                        )
                        o_t = outp.tile([P, 512], f32, name="o_t")
                        nc.scalar.activation(
                            o_t[:, :w],
                            ps2_t[:, :w],
                            mybir.ActivationFunctionType.Relu,
                            bias=b_t[:],
                        )
                        nc.sync.dma_start(out_d[:, col : col + w], o_t[:, :w])
                        b0 += nb

                icol = 0
                ipiece = 0
                qn = 0
                for g, (b0g, nbg) in enumerate(groups):
                    ps_ts = [
                        psp.tile([P, W129], f32, name="ps_t") for bi in range(nbg)
                    ]
                    for k in range(L):
                        cap = caps[g][k]
                        pl = pieces[g][k]
                        np_k = len(pl)
                        g_t = gatp.tile([P, capmax * 2], fp8)
                        g3 = g_t[:].rearrange("p (c d) -> p c d", d=2 * D)
                        ni = nidx[g][k]
                        nc.gpsimd.dma_gather(
                            out_ap=g3[:, 0 : -(-ni // 128), :],
                            in_ap=tabs_d[k][:],
                            idxs_ap=idx_t[:, icol : icol + (-(-ni // 16))],
                            num_idxs=ni,
                            num_idxs_reg=ni,
                            elem_size=2 * D,
                            single_packet=False,
                            queue_num=qn,
                        )
                        icol += cap // 16
                        qn = (qn + 1) % 4
                        if np_k == 0:
                            continue
                        s_t = sp_.tile([P, np_k * W130], bf16)
                        relp_b = (
                            relp_t[:, ipiece * 2 : (ipiece + np_k) * 2]
                            .rearrange("p (c t) -> p c t", t=2)
                            .unsqueeze(2)
                            .broadcast_to([P, np_k, W130 // 2, 2])
                        )
                        iota_b = iota_ab.unsqueeze(1).broadcast_to(
                            [P, np_k, W130 // 2, 2]
                        )
                        s_b = s_t[:].rearrange(
                            "p (c a b) -> p c a b", a=W130 // 2, b=2
                        )
                        nc.vector.tensor_tensor(
                            out=s_b, in0=iota_b, in1=relp_b, op=mybir.AluOpType.is_lt
                        )
                        s3 = s_t[:].rearrange("p (c w) -> p c w", w=W130)
                        for pi, (ch, half, bi, start, stop) in enumerate(pl):
                            nc.tensor.matmul(
                                out=ps_ts[bi][:],
                                lhsT=g_t[
                                    :,
                                    ch * 2 * D + half * D : ch * 2 * D + half * D + D,
                                ],
                                rhs=s3[:, pi, 0:W129],
                                start=bool(start),
                                stop=bool(stop),
                            )
                        ipiece += np_k
                    for bi in range(nbg):
                        nc.scalar.activation(
                            out=buf_t[:, (b0g + bi) * W129 : (b0g + bi + 1) * W129],
                            in_=ps_ts[bi][:],
                            func=mybir.ActivationFunctionType.Copy,
                        )
                    if g == 8:
                        # early phase 2 for long-ready groups: inputs
                        # finished ~2 groups ago, so no engine-queue
                        # head-of-line waits; shrinks the end tail
                        phase2(0, P2_SPLIT)
                    elif g == 11:
                        phase2(P2_SPLIT, 40)

                # Phase 2 for remaining blocks
                phase2(40, NB)

    nc.compile()
    return nc


def _prep_inputs(h, edge_index, W_msg, W_upd, b_upd):
    """Host prep: per-core edge bucketing, L matchings -> pair tables,
    static stream/piece plan shared across cores."""
    import ml_dtypes

    N0, d = h.shape
    assert d == D
    E = edge_index.shape[1]

    SP = -(-N0 // (N_CORES * P)) * P
    NB = SP // P
    NWIN = N0 // 2
    PADW = NWIN  # zero pad window index
    NW = NWIN + 1
    L = L_COPIES

    src = np.ascontiguousarray(edge_index[0]).astype(np.int64)
    dst = np.ascontiguousarray(edge_index[1]).astype(np.int64)
    gblock = dst >> 7
    core_of = np.minimum(gblock // NB, N_CORES - 1)
    slot_of = gblock - core_of * NB
    rel = (dst & 127).astype(np.int64)

    groups = []
    b0 = 0
    while b0 < NB:
        nb = min(GB, NB - b0)
        groups.append((b0, nb))
        b0 += nb
    NG = len(groups)
    group_of_slot = np.zeros(NB, np.int64)
    for gi, (b0g, nbg) in enumerate(groups):
        group_of_slot[b0g : b0g + nbg] = gi

    rng = np.random.default_rng(12345)
    hbf = h.astype(ml_dtypes.bfloat16)
    h8 = h.astype(ml_dtypes.float8_e3m4)

    # entries[c][g][k][slot] = list of (window, relA, relB)
    entries = [
        [[[[] for _ in range(NB)] for _ in range(L)] for _ in range(NG)]
        for _ in range(N_CORES)
    ]
    tables = [[None] * L for _ in range(N_CORES)]

    for c in range(N_CORES):
        m = np.flatnonzero(core_of == c)
        es = src[m]
        eslot = slot_of[m]
        erel = rel[m]
        covered = np.zeros(len(es), bool)
        pos_maps = []
        for k in range(L):
            ridx = np.flatnonzero(~covered)
            o = np.lexsort((es[ridx], eslot[ridx]))
            r = ridx[o]
            rb = eslot[r]
            same = rb[:-1] == rb[1:]
            newrun = np.r_[True, rb[1:] != rb[:-1]]
            pos = np.arange(len(r)) - np.maximum.accumulate(
                np.where(newrun, np.arange(len(r)), 0)
            )
            i_idx = np.flatnonzero((pos[:-1] % 2 == 0) & same)
            e1 = r[i_idx]
            e2 = r[i_idx + 1]
            s1 = es[e1]
            s2 = es[e2]
            v = s1 != s2
            e1, e2, s1, s2 = e1[v], e2[v], s1[v], s2[v]
            taken = np.zeros(N0, bool)
            acc_e1, acc_e2 = [], []
            remaining = rng.permutation(len(e1))
            for _ in range(4):
                if len(remaining) == 0:
                    break
                aa, bb = s1[remaining], s2[remaining]
                ok = ~taken[aa] & ~taken[bb]
                cand = remaining[ok]
                if len(cand) == 0:
                    break
                aa, bb = s1[cand], s2[cand]
                fa = np.zeros(len(cand), bool)
                fb = np.zeros(len(cand), bool)
                _, fi = np.unique(aa, return_index=True)
                fa[fi] = True
                _, fi2 = np.unique(bb, return_index=True)
                fb[fi2] = True
                acc = np.flatnonzero(fa & fb)
                s1a, s2a = aa[acc], bb[acc]
                seen2 = np.zeros(N0, bool)
                keep = np.zeros(len(acc), bool)
                for i_ in range(len(acc)):
                    x, y = s1a[i_], s2a[i_]
                    if not (seen2[x] or seen2[y]):
                        keep[i_] = True
                        seen2[x] = True
                        seen2[y] = True
                accepted = cand[acc[keep]]
                taken[s1[accepted]] = True
                taken[s2[accepted]] = True
                acc_e1.append(e1[accepted])
                acc_e2.append(e2[accepted])
                covered[e1[accepted]] = True
                covered[e2[accepted]] = True
                remaining = remaining[~taken[s1[remaining]] & ~taken[s2[remaining]]]
            pe1 = np.concatenate(acc_e1) if acc_e1 else np.empty(0, np.int64)
            pe2 = np.concatenate(acc_e2) if acc_e2 else np.empty(0, np.int64)
            ps1, ps2 = es[pe1], es[pe2]
            used = np.zeros(N0, bool)
            used[ps1] = True
            used[ps2] = True
            restn = np.flatnonzero(~used)
            perm = np.empty(N0, np.int64)
            npair = len(ps1)
            perm[0 : 2 * npair : 2] = ps1
            perm[1 : 2 * npair : 2] = ps2
            perm[2 * npair :] = restn
            pos_of = np.empty(N0, np.int64)
            pos_of[perm] = np.arange(N0)
            pos_maps.append(pos_of)
            tab = np.zeros((NW, 2 * D), dtype=ml_dtypes.float8_e3m4)
            tab[:NWIN] = h8[perm].reshape(NWIN, 2 * D)
            tables[c][k] = tab
            sl_arr = eslot[pe1]
            ra_arr = erel[pe1]
            rb_arr = erel[pe2]
            for j in range(npair):
                sl = int(sl_arr[j])
                entries[c][group_of_slot[sl]][k][sl].append(
                    (j, int(ra_arr[j]), int(rb_arr[j]))
                )
        # singles: per-bucket water-fill over copies so the 6 sections of
        # each (core, block) end up near-equal -> smaller cross-core max
        sing = np.flatnonzero(~covered)
        so = sing[np.argsort(eslot[sing], kind="stable")]
        cur_sl = -1
        counts = [0] * L
        for ei in so:
            sl = int(eslot[ei])
            if sl != cur_sl:
                cur_sl = sl
                gg = group_of_slot[sl]
                counts = [len(entries[c][gg][k][sl]) for k in range(L)]
            k = min(range(L), key=counts.__getitem__)
            counts[k] += 1
            p = int(pos_maps[k][es[ei]])
            w = p >> 1
            if p & 1 == 0:
                entries[c][group_of_slot[sl]][k][sl].append((w, int(erel[ei]), -1))
            else:
                entries[c][group_of_slot[sl]][k][sl].append((w, -1, int(erel[ei])))

    # static caps per (g, k, block): max over cores, mult of 16
    capgkb = np.zeros((NG, L, NB), np.int64)
    for g in range(NG):
        for k in range(L):
            for bsl in range(NB):
                mx = max(len(entries[c][g][k][bsl]) for c in range(N_CORES))
                capgkb[g, k, bsl] = -(-mx // 16) * 16
    caps = []
    for g in range(NG):
        ck = []
        for k in range(L):
            t = int(capgkb[g, k, :].sum())
            ck.append(-(-max(t, 128) // 128) * 128)
        caps.append(ck)
    # exact stream lengths; the first 8 streams (one per gather-pool ring
    # slot) gather the max chunk span any later stream on their slot reads,
    # so no stale tile tail is ever read under a rel=-1 staircase row
    flat_t = [int(capgkb[g, k, :].sum()) for g in range(NG) for k in range(L)]
    warm = [128] * 10
    for s, t in enumerate(flat_t):
        r = s % 10
        warm[r] = max(warm[r], -(-t // 128) * 128)
    nidx = []
    si = 0
    for g in range(NG):
        nk = []
        for k in range(L):
            t = int(capgkb[g, k, :].sum())
            if si < 10:
                caps[g][k] = max(warm[si], -(-max(t, 128) // 128) * 128)
                nk.append(caps[g][k])
            else:
                nk.append(max(t, 16))
            si += 1
        nidx.append(nk)

    # static sections and piece lists
    stream_sections = [[None] * L for _ in range(NG)]
    pieces = [[None] * L for _ in range(NG)]
    for g, (b0g, nbg) in enumerate(groups):
        for k in range(L):
            secs = []
            off = 0
            for bi in range(nbg):
                sl = b0g + bi
                cb = int(capgkb[g, k, sl])
                if cb:
                    secs.append((off, off + cb, bi, sl))
                off += cb
            stream_sections[g][k] = secs
            cap = caps[g][k]
            pl = []
            for ch in range(cap // 128):
                lo_c, hi_c = ch * 128, (ch + 1) * 128
                for off_lo, off_hi, bi, sl in secs:
                    a = max(lo_c, off_lo)
                    b_ = min(hi_c, off_hi)
                    if a < b_:
                        for half in (0, 1):
                            pl.append([ch, half, bi, 0, 0, a - lo_c, b_ - lo_c, sl])
            pieces[g][k] = pl
    # start/stop flags in program emission order
    seen_first = set()
    last_ref = {}
    for g in range(NG):
        for k in range(L):
            for p in pieces[g][k]:
                sl = p[7]
                if sl not in seen_first:
                    p[3] = 1
                    seen_first.add(sl)
                last_ref[sl] = p
    for sl, p in last_ref.items():
        p[4] = 1
    assert len(seen_first) == NB, (len(seen_first), NB)

    npieces = sum(len(pieces[g][k]) for g in range(NG) for k in range(L))
    idx_cols = sum(caps[g][k] // 16 for g in range(NG) for k in range(L))

    w1T = np.ascontiguousarray(
        W_upd[:, :D].T.astype(np.float32).astype(ml_dtypes.bfloat16)
    )
    wc = (W_upd[:, D:].astype(np.float64) @ W_msg.astype(np.float64)).astype(
        np.float32
    )
    wcT = np.ascontiguousarray(wc.T.astype(ml_dtypes.bfloat16))
    bias = np.ascontiguousarray(b_upd.astype(np.float32).reshape(P, 1))
    iota = np.ascontiguousarray(
        np.tile(np.arange(W130, dtype=np.float32), (P, 1)).astype(ml_dtypes.bfloat16)
    )

    in_maps = []
    for c in range(N_CORES):
        idx_flat = np.full(idx_cols * 16, PADW, np.int16)
        relp_arr = np.full((P, npieces * 2), -0.5, np.float32)
        ioff = 0
        poff = 0
        for g, (b0g, nbg) in enumerate(groups):
            for k in range(L):
                cap = caps[g][k]
                secs = stream_sections[g][k]
                slots_w = np.full(cap, PADW, np.int64)
                slots_rA = np.full(cap, -1.0, np.float32)
                slots_rB = np.full(cap, -1.0, np.float32)
                for off_lo, off_hi, bi, sl in secs:
                    ent = entries[c][g][k][sl]
                    n = len(ent)
                    if n:
                        slots_w[off_lo : off_lo + n] = [e[0] for e in ent]
                        slots_rA[off_lo : off_lo + n] = [e[1] for e in ent]
                        slots_rB[off_lo : off_lo + n] = [e[2] for e in ent]
                idx_flat[ioff : ioff + cap] = slots_w.astype(np.int16)
                ioff += cap
                for p in pieces[g][k]:
                    ch, half, bi, st, sp2, a, b_, sl = p
                    vals = np.full(P, -0.5, np.float32)
                    base = ch * 128
                    rr = (slots_rA if half == 0 else slots_rB)[base + a : base + b_]
                    vals[a:b_] = rr + 0.5
                    relp_arr[:, poff * 2] = vals
                    relp_arr[:, poff * 2 + 1] = vals
                    poff += 1
        assert ioff == idx_cols * 16 and poff == npieces
        idx16 = idx_flat.reshape(idx_cols, 16).T  # [16, cols]
        idx_in = np.tile(idx16, (8, 1))

        lo = c * SP
        hi = min((c + 1) * SP, N0)
        hs = np.zeros((SP, D), dtype=ml_dtypes.bfloat16)
        if hi > lo:
            hs[: hi - lo] = hbf[lo:hi]
        im = {
            "hsT": np.ascontiguousarray(hs.T),
            "idx": np.ascontiguousarray(idx_in),
            "relp": np.ascontiguousarray(relp_arr.astype(ml_dtypes.bfloat16)),
            "iota": iota,
            "w1T": w1T,
            "wcT": wcT,
            "bias": bias,
        }
        for k in range(L):
            im[f"tab{k}"] = tables[c][k]
        in_maps.append(im)

    plan = {
        "NB": NB,
        "NW": NW,
        "caps": caps,
        "nidx": nidx,
        "pieces": [
            [[(p[0], p[1], p[2], p[3], p[4]) for p in pieces[g][k]] for k in range(L)]
            for g in range(NG)
        ],
        "npieces": npieces,
        "groups": groups,
        "idx_cols": idx_cols,
    }
    return in_maps, plan, SP, NB


def kernel_with_results(h, edge_index, W_msg, W_upd, b_upd, loop_iters=None, **run_kwargs):
    in_maps, plan, SP, NB = _prep_inputs(h, edge_index, W_msg, W_upd, b_upd)

    key = (
        plan["NB"],
        plan["NW"],
        tuple(tuple(ck) for ck in plan["caps"]),
        tuple(
            tuple(map(tuple, plan["pieces"][g][k]))
            for g in range(len(plan["caps"]))
            for k in range(L_COPIES)
        ),
        loop_iters,
    )
    if key not in _prog_cache:
        _prog_cache[key] = _build_program(plan, loop_iters=loop_iters)
    nc = _prog_cache[key]

    res = run_bass_kernel_spmd(nc, in_maps, core_ids=list(range(N_CORES)), **run_kwargs)

    N0 = h.shape[0]
    out = np.empty((N0, D), dtype=np.float32)
    for c in range(N_CORES):
        lo = c * SP
        hi = min((c + 1) * SP, N0)
        if hi > lo:
            out[lo:hi] = res.results[c]["outT"].T[: hi - lo]
    return out, res


def kernel(h, edge_index, W_msg, W_upd, b_upd):
    out, _ = kernel_with_results(h, edge_index, W_msg, W_upd, b_upd)
    return out
